# revision 1
# baseline (speedup 1.0000x reference)
"""CapsNet dynamic-routing layer on 8 Trainium2 NeuronCores.

Sharding: tensor-parallel over num_caps_j (J=32 -> 4 per core); every
(batch, j) routing chain is core-local, so there are no collectives.

Layout: the einsum runs W-stationary / x-moving, so u_hat lands as
U[(j,v); b, i] (partitions = the 128 (j,v) pairs of this core's shard).
That layout lets the routing iteration split across engines:
  - t-pass product u*w on the DVE at 2x (w pair-expanded so the
    broadcast is non-innermost), then the v-reduction as a TensorE
    matmul against a block-diagonal indicator: PSUM rows come out
    replicated over v, which is exactly the operand shape the s-pass
    needs.
  - exp on the scalar engine straight from PSUM, with accum_out
    producing the softmax denominator per (b, j) for free.
  - s-pass product + i-fold on the DVE at 2x.
Squash norms (sum over v = partitions) also go through the indicator
matmul on TensorE. s0 = sum_i u_hat is folded on the DVE while the
einsum streams (DVE is idle there).
"""

import sys

if "/opt/trn_rl_repo" not in sys.path:
    sys.path.insert(0, "/opt/trn_rl_repo")

import numpy as np

B, I, D, J, V = 128, 512, 256, 32, 32
NCORES = 8
JL = J // NCORES          # 4 j's per core
JV = JL * V               # 128 partitions = (j, v)
DP = 128                  # contraction chunk (partitions)
EPS = 1e-9
IBLK = 16                 # i-block per DMA tile / PSUM tile
TB = 8                    # routing tile: b's per tile
NTB = B // TB             # 16 tiles
PB = 2                    # b's per PE reduce chunk (N = PB*I = 1024)

_cache = {}


def _build_program():
    import concourse.tile as tile
    from concourse import bacc, mybir

    f16 = mybir.dt.float16
    f32 = mybir.dt.float32
    MULT = mybir.AluOpType.mult
    EXP = mybir.ActivationFunctionType.Exp
    SQRT = mybir.ActivationFunctionType.Sqrt

    nc = bacc.Bacc("TRN2", target_bir_lowering=False, debug=False,
                   num_devices=NCORES)

    xa = nc.dram_tensor("xa", [DP, I, B], f16, kind="ExternalInput")
    xb = nc.dram_tensor("xb", [DP, I, B], f16, kind="ExternalInput")
    wa = nc.dram_tensor("wa", [DP, I, JV], f16, kind="ExternalInput")
    wb = nc.dram_tensor("wb", [DP, I, JV], f16, kind="ExternalInput")
    ind = nc.dram_tensor("ind", [128, 128], f16, kind="ExternalInput")
    v2d = nc.dram_tensor("v2", [JV, B], f32, kind="ExternalOutput")

    with tile.TileContext(nc) as tc:
        from contextlib import ExitStack
        stack = ExitStack()
        upool = stack.enter_context(tc.tile_pool(name="uhat", bufs=1))
        pspool = stack.enter_context(
            tc.tile_pool(name="psum", bufs=2, space="PSUM"))
        rpool = stack.enter_context(tc.tile_pool(name="rout", bufs=1))
        # one shared pool: einsum staging tiles (phase 1) and routing
        # tiles (phase 2) rotate through the same four tags
        bigpool = stack.enter_context(tc.tile_pool(name="big", bufs=2))

        eps_t = rpool.tile([128, 1], f32, tag="eps")
        nc.gpsimd.memset(eps_t[:], EPS)

        ind_t = rpool.tile([128, 128], f16, tag="ind")
        nc.sync.dma_start(ind_t[:], ind.ap())

        # u_hat, [(j,v); b, i] fp16, 16.8 MB
        U = upool.tile([128, B, I], f16)
        s0 = rpool.tile([128, B], f32, tag="s0")

        def fold_i(prod, nb, n, out_ap):
            """Fold [128, nb, n] fp16 over the innermost i axis down to
            out_ap [128, nb]. All levels rotate through one 4KB tag."""
            cur = prod
            while n > 2:
                nh = n // 2
                nxt = bigpool.tile([128, 2048], f16, tag="fld")
                nv = nxt[:, 0:nb * nh].rearrange("p (b i) -> p b i", i=nh)
                nc.vector.tensor_add(nv, cur[:, :, 0:nh], cur[:, :, nh:n])
                cur = nv
                n = nh
            nc.vector.tensor_add(out_ap, cur[:, :, 0], cur[:, :, 1])

        # ---- einsum: u_hat[(j,v); b, i] = sum_d W[d,(j,v),i] x[d,b,i] --
        # 1MB DMA tiles; the last two blocks taper to 16 i's so the
        # post-DMA tail (MMs -> copy -> s0 fold -> squash) is shorter.
        blocks = [(k * 32, 32) for k in range(15)] + [(480, 16), (496, 16)]
        for blk, (i0, DBLK) in enumerate(blocks):
            xa_t = bigpool.tile([128, DBLK, B], f16, tag="xa")
            nc.sync.dma_start(xa_t[:], xa.ap()[:, i0:i0 + DBLK, :])
            xb_t = bigpool.tile([128, DBLK, B], f16, tag="xb")
            nc.sync.dma_start(xb_t[:], xb.ap()[:, i0:i0 + DBLK, :])
            wa_t = bigpool.tile([128, DBLK, JV], f16, tag="wa")
            nc.gpsimd.dma_start(wa_t[:], wa.ap()[:, i0:i0 + DBLK, :])
            wb_t = bigpool.tile([128, DBLK, JV], f16, tag="wb")
            nc.gpsimd.dma_start(wb_t[:], wb.ap()[:, i0:i0 + DBLK, :])

            for sb in range(DBLK // IBLK):
                k0 = sb * IBLK
                ps = pspool.tile([JV, IBLK, B], f32, tag="ps")  # 4 banks
                for k in range(k0, k0 + IBLK):
                    nc.tensor.matmul(
                        ps[:, k - k0, :], wa_t[:, k, :], xa_t[:, k, :],
                        start=True, stop=False)
                    nc.tensor.matmul(
                        ps[:, k - k0, :], wb_t[:, k, :], xb_t[:, k, :],
                        start=False, stop=True)
                # scatter into [(j,v); b, i]
                nc.scalar.copy(
                    U[:, :, i0 + k0:i0 + k0 + IBLK],
                    ps.rearrange("p i b -> p b i"))
            # s0 partial folds while the PE streams (32-i chunks)
            s0p = rpool.tile([128, B], f32, tag="s0p")
            fold_i(U[:, :, i0:i0 + DBLK], B, DBLK, s0p[:])
            if blk == 0:
                nc.vector.tensor_copy(s0[:], s0p[:])
            else:
                nc.vector.tensor_add(s0[:], s0[:], s0p[:])

        # ---- routing state ----------------------------------------
        w2 = rpool.tile([128, B], f16, tag="w2")      # cumulative sum of v
        w2p = rpool.tile([128, B, 2], f16, tag="w2p")  # pair-expanded
        Zbuf = rpool.tile([128, B], f32, tag="Zbuf")
        Zrec = rpool.tile([128, B], f32, tag="Zrec")
        s_un = rpool.tile([128, B], f32, tag="s_un")
        ssc = rpool.tile([128, B], f32, tag="ssc")
        sq = rpool.tile([128, B], f16, tag="sq")
        d1 = rpool.tile([128, B], f32, tag="d1")
        r1 = rpool.tile([128, B], f32, tag="r1")
        rt = rpool.tile([128, B], f32, tag="rt")
        r2 = rpool.tile([128, B], f32, tag="r2")
        fac = rpool.tile([128, B], f32, tag="fac")
        vout = rpool.tile([128, B], f32, tag="vout")

        def squash(s_ap, v_ap):
            # v = s * n2/((1+n2)*sqrt(n2+EPS)); n2 = sum_v s^2 over the
            # 32 v-partitions of each j block, via the indicator matmul.
            # Single reciprocal of the combined denominator.
            nc.vector.tensor_mul(sq[:], s_ap, s_ap)
            psN = pspool.tile([128, B], f32, tag="ps")
            nc.tensor.matmul(psN[:], ind_t[:], sq[:], start=True, stop=True)
            nc.scalar.add(d1[:], psN[:], 1.0)
            nc.scalar.activation(rt[:], psN[:], SQRT, bias=eps_t[:])
            nc.vector.tensor_mul(r1[:], d1[:], rt[:])
            nc.vector.reciprocal(r2[:], r1[:])
            nc.vector.tensor_tensor(fac[:], psN[:], r2[:], op=MULT)
            nc.vector.tensor_tensor(v_ap, s_ap, fac[:], op=MULT)

        def set_w(v_ap, first):
            if first:
                nc.vector.tensor_copy(w2[:], v_ap)
            else:
                nc.vector.tensor_add(w2[:], w2[:], v_ap)
            nc.vector.tensor_copy(w2p[:, :, 0], w2[:])
            nc.vector.tensor_copy(w2p[:, :, 1], w2[:])

        squash(s0[:], vout[:])
        set_w(vout[:], first=True)

        # ---- 2 routing iterations ---------------------------------
        for r in (1, 2):
            def t_prod(tb):
                bsl = slice(tb * TB, (tb + 1) * TB)
                pr = bigpool.tile([128, TB, I], f16, tag="xa")
                uv = U[:, bsl, :].rearrange("p b (i two) -> p b i two",
                                            two=2)
                wv = (w2p[:, bsl, :].unsqueeze(2)
                      .broadcast_to([128, TB, I // 2, 2]))
                prv = pr[:].rearrange("p b (i two) -> p b i two", two=2)
                nc.vector.tensor_tensor(prv, uv, wv, op=MULT)
                return pr

            prs = {0: t_prod(0)}
            for tb in range(NTB):
                if tb + 1 < NTB:
                    prs[tb + 1] = t_prod(tb + 1)
                pr = prs.pop(tb)
                bsl = slice(tb * TB, (tb + 1) * TB)
                # PE v-reduce -> exp (+ softmax denominator)
                c2 = bigpool.tile([128, TB, I], f16, tag="xb")
                for cb in range(TB // PB):
                    psB = pspool.tile([128, PB, I], f32, tag="ps")
                    for m in range(PB):
                        nc.tensor.matmul(psB[:, m, :], ind_t[:],
                                         pr[:, cb * PB + m, :],
                                         start=True, stop=True)
                    for m in range(PB):
                        bi = tb * TB + cb * PB + m
                        nc.scalar.activation(
                            c2[:, cb * PB + m, :], psB[:, m, :], EXP,
                            accum_out=Zbuf[:, bi:bi + 1])
                # s-pass product + fold over i
                prs2 = bigpool.tile([128, TB, I], f16, tag="wa")
                nc.vector.tensor_tensor(prs2[:], U[:, bsl, :], c2[:],
                                        op=MULT)
                fold_i(prs2[:], TB, I, s_un[:, bsl])

            # s = s_un * (I / Z), then squash.  The Zbuf->Zrec hop runs
            # on the scalar engine, whose FIFO order guarantees it sees
            # every accum_out column the exps above produced.
            nc.scalar.copy(Zrec[:], Zbuf[:])
            nc.vector.reciprocal(Zrec[:], Zrec[:])
            nc.vector.tensor_tensor(ssc[:], s_un[:], Zrec[:], op=MULT)
            nc.scalar.mul(ssc[:], ssc[:], float(I))
            squash(ssc[:], vout[:])
            if r == 1:
                set_w(vout[:], first=False)
            else:
                nc.sync.dma_start(v2d.ap(), vout[:])
        stack.close()

    nc.compile()
    return nc


def _get_program():
    if "nc" not in _cache:
        _cache["nc"] = _build_program()
    return _cache["nc"]


def _prep_inputs(x, W):
    """Host-side shard + transpose + fp16 cast."""
    u = np.ascontiguousarray(x[..., 0])                   # [B, I, D] f32
    xt = np.ascontiguousarray(u.transpose(2, 1, 0)).astype(np.float16)
    xa_np = np.ascontiguousarray(xt[:DP])                 # [128, I, B]
    xb_np = np.ascontiguousarray(xt[DP:])
    ind_np = np.kron(np.eye(JL, dtype=np.float16),
                     np.ones((V, V), dtype=np.float16))   # [128, 128]
    W0 = W[0]                                             # [I, J, D, V]
    in_maps = []
    for c in range(NCORES):
        Wc = W0[:, c * JL:(c + 1) * JL]                   # [I, JL, D, V]
        Wt = Wc.transpose(2, 0, 1, 3)                     # [D, I, JL, V]
        Wt = Wt.reshape(D, I, JV).astype(np.float16)
        in_maps.append({
            "xa": xa_np,
            "xb": xb_np,
            "wa": np.ascontiguousarray(Wt[:DP]),
            "wb": np.ascontiguousarray(Wt[DP:]),
            "ind": ind_np,
        })
    return in_maps


def run_cores(x, W, trace=False):
    from concourse import bass_utils
    nc = _get_program()
    in_maps = _prep_inputs(x, W)
    res = bass_utils.run_bass_kernel_spmd(
        nc, in_maps, core_ids=list(range(NCORES)), trace=trace)
    return res


def kernel(x, W):
    x = np.asarray(x)
    W = np.asarray(W)
    res = run_cores(x, W, trace=False)
    out = np.empty((B, J, V, 1), dtype=np.float32)
    for c in range(NCORES):
        vc = res.results[c]["v2"].reshape(JL, V, B)       # [(j,v); b]
        out[:, c * JL:(c + 1) * JL, :, 0] = vc.transpose(2, 0, 1)
    return out

